# revision 36
# baseline (speedup 1.0000x reference)
"""Trainium2 Bass kernel for per-attribute MLP decoder (nn_AttrDecoder).

Computes, for each attribute a (A=312 independent blocks):
    h = relu(x[:, a*64:(a+1)*64] @ W1[a] + b1[a])      # [B, 128]
    o[:, a] = sigmoid(h @ W2[a] + b2[a])               # [B, 1]

Strategy:
  - Data-parallel over batch: B=8192 -> 1024 rows per core across 8 cores.
  - x is marshaled on the host to bf16 and transposed ([A*LAT, B]) so the
    contraction dim (LAT) lands on SBUF partitions as the tensor engine
    requires, and weights are pre-packed into PE-friendly layouts.
  - MM1: attributes in pairs; W1[2i] on PE rows 0-63, W1[2i+1] on rows
    64-127; two row-tiled K=64 matmuls run concurrently (tile_position
    (0,0)/(64,0)), N=512 batch columns, h^T accumulated in PSUM.
  - ReLU + b1 fused into the PSUM->SBUF copy (bf16 out), split across
    ScalarE (activation) and VectorE (tensor_scalar add+max) by throughput.
  - MM2: per attribute quad, four M=1 col-tiled matmuls (tile_position
    (0,32j), PSUM partitions {0,32,64,96} of a shared bank) so the four
    h-streams overlap on different PE column-groups.
  - Sigmoid + b2 on ScalarE per quad bank; strided-partition DMA stores
    rows 4q..4q+3 of the o^T output; host transposes during the gather.
"""

import numpy as np
import ml_dtypes

import concourse.bass as bass
import concourse.tile as tile
from concourse import mybir
from concourse import bass_utils

A = 312
LAT = 64
HID = 128
B = 8192
NCORES = 8
BS = B // NCORES          # 1024 batch rows per core
NPAIR = A // 2            # 156
NQUAD = A // 4            # 78
BT = 512                  # batch tile (one PSUM bank of fp32)
NBT = BS // BT            # 2

_cached = {}


def _legalize_waits(nc, max_waits=1):
    """Walrus in this toolchain encodes at most one sync-wait per instruction.
    Hoist extra waits onto standalone EventSemaphore instructions placed just
    before the owner on the same engine queue (queue order preserves the
    happens-before)."""
    nsplit = 0
    for bb in nc.m.functions[0].blocks:
        new_insts = []
        changed = False
        for inst in bb.instructions:
            si = getattr(inst, "sync_info", None)
            if si is not None and len(si.on_wait) > max_waits:
                waits = list(si.on_wait)
                for k, w in enumerate(waits[:-max_waits]):
                    es = mybir.InstEventSemaphore(name=f"{inst.name}-hw{k}")
                    es.engine = inst.engine
                    es.opcode = "EventSemaphore"
                    es.sync_info = mybir.SyncInfo(on_wait=[w], on_update=[])
                    new_insts.append(es)
                    nsplit += 1
                inst.sync_info = mybir.SyncInfo(
                    on_wait=waits[-max_waits:], on_update=list(si.on_update))
                changed = True
            new_insts.append(inst)
        if changed:
            bb.instructions = new_insts
    return nsplit


def _build_nc():
    nc = bass.Bass("TRN2", target_bir_lowering=False, debug=False,
                   num_devices=NCORES)
    xt = nc.dram_tensor("xt", [A * LAT, BS], mybir.dt.bfloat16,
                        kind="ExternalInput").ap()
    w1 = nc.dram_tensor("w1", [128, NPAIR, 128], mybir.dt.bfloat16,
                        kind="ExternalInput").ap()
    w2 = nc.dram_tensor("w2", [HID, A], mybir.dt.bfloat16,
                        kind="ExternalInput").ap()
    b1 = nc.dram_tensor("b1", [HID, A], mybir.dt.float32,
                        kind="ExternalInput").ap()
    b2 = nc.dram_tensor("b2", [128, NQUAD], mybir.dt.float32,
                        kind="ExternalInput").ap()
    ot = nc.dram_tensor("ot", [A, BS], mybir.dt.float32,
                        kind="ExternalOutput").ap()

    with tile.TileContext(nc, trace_sim=False) as tc:
        _body(tc, xt, w1, w2, b1, b2, ot)
    _legalize_waits(nc)
    return nc


def _body(tc, xt, w1, w2, b1, b2, ot):
    nc = tc.nc
    from contextlib import ExitStack
    with ExitStack() as ctx:
        singles = ctx.enter_context(tc.tile_pool(name="singles", bufs=1))
        xpool = ctx.enter_context(tc.tile_pool(name="x", bufs=6))
        hpool_a = ctx.enter_context(tc.tile_pool(name="ha", bufs=6))
        hpool_v = ctx.enter_context(tc.tile_pool(name="hv", bufs=6))
        opool = ctx.enter_context(tc.tile_pool(name="osb", bufs=3))
        hps_a = ctx.enter_context(
            tc.tile_pool(name="hpsa", bufs=1, space=bass.MemorySpace.PSUM))
        hps_v = ctx.enter_context(
            tc.tile_pool(name="hpsv", bufs=2, space=bass.MemorySpace.PSUM))
        ops = ctx.enter_context(
            tc.tile_pool(name="ops", bufs=1, space=bass.MemorySpace.PSUM))

        # Resident weights/biases. w1 is 5MB: load in chunks so the first
        # pairs' matmuls don't stall on the whole transfer.
        w1_sb = singles.tile([128, NPAIR, 128], mybir.dt.bfloat16)
        W1CH = 12  # first chunk split below
        nc.scalar.dma_start(w1_sb[:, 0:4, :], w1[:, 0:4, :])
        for c in range(4, NPAIR, W1CH):
            ce = min(c + W1CH, NPAIR)
            nc.scalar.dma_start(w1_sb[:, c:ce, :], w1[:, c:ce, :])
        w2_sb = singles.tile([HID, A], mybir.dt.bfloat16)
        nc.scalar.dma_start(w2_sb[:], w2[:])
        b1_sb = singles.tile([HID, A], mybir.dt.float32)
        nc.scalar.dma_start(b1_sb[:], b1[:])
        b2_sb = singles.tile([128, NQUAD], mybir.dt.float32)
        nc.scalar.dma_start(b2_sb[:], b2[:])

        o_ps = ops.tile([128, NBT, BT], mybir.dt.float32, name="o_bank")

        def mm2_only(quad):
            """Emit one quad's 8 MM2s, bt-outer so the 4 attrs' streams hit
            4 different PE column-groups back-to-back (concurrent)."""
            for bt in range(NBT):
                for a, h_sb in quad:
                    jj = a % 4
                    nc.tensor.matmul(
                        o_ps[32 * jj:32 * jj + 1, bt, :],
                        w2_sb[:, a:a + 1],
                        h_sb[:, bt, :],
                        start=True, stop=True,
                        tile_position=(0, 32 * jj),
                    )

        def drain(q):
            """Sigmoid + store for quad q, emitted a full pair after its
            MM2s so the sigmoid never blocks the ACT queue on them."""
            o_sb = opool.tile([128, NBT, BT], mybir.dt.float32, name="osb")
            nc.scalar.activation(
                out=o_sb[:], in_=o_ps[:],
                func=mybir.ActivationFunctionType.Sigmoid,
                bias=b2_sb[:, q:q + 1], scale=1.0)
            nc.sync.dma_start(
                out=ot[4 * q:4 * q + 4, :].rearrange(
                    "p (n b) -> p n b", n=NBT),
                in_=o_sb[::32, :, :])

        pend = []           # (a, h_sb) entries not yet MM2'd
        sig_q = None        # quad whose sigmoid/store is deferred
        for p in range(NPAIR):
            x_tile = xpool.tile([128, BS], mybir.dt.bfloat16)
            nc.sync.dma_start(out=x_tile[:],
                              in_=xt[p * 128:(p + 1) * 128, :])
            uact = [(2 * p + j) % 2 == 0 and (2 * p + j) % 12 != 0
                    for j in range(2)]
            h_pss = [
                (hps_a if uact[j] else hps_v).tile(
                    [128, NBT, BT], mybir.dt.float32,
                    name="hpsa" if uact[j] else "hpsv")
                for j in range(2)]
            # MM1s: interleave the two attrs (disjoint PE row groups) so
            # their streams overlap; bt-halves of one attr are sequential.
            for bt in range(NBT):
                for j in range(2):
                    nc.tensor.matmul(
                        h_pss[j][:, bt, :],
                        w1_sb[j * 64:(j + 1) * 64, p, :],
                        x_tile[j * 64:(j + 1) * 64, bass.ds(bt * BT, BT)],
                        start=True, stop=True,
                        tile_position=(j * 64, 0),
                    )
            # relu: one FD=1024 op per attr, alternating engines
            new_pend = []
            for j in range(2):
                a = 2 * p + j
                use_act = uact[j]
                hp = hpool_a if use_act else hpool_v
                h_sb = hp.tile([HID, NBT, BT], mybir.dt.bfloat16,
                               name="hsb")
                if use_act:
                    nc.scalar.activation(
                        out=h_sb[:], in_=h_pss[j][:],
                        func=mybir.ActivationFunctionType.Relu,
                        bias=b1_sb[:, a:a + 1], scale=1.0)
                else:
                    nc.vector.tensor_scalar(
                        out=h_sb[:], in0=h_pss[j][:],
                        scalar1=b1_sb[:, a:a + 1], scalar2=0.0,
                        op0=mybir.AluOpType.add,
                        op1=mybir.AluOpType.max)
                new_pend.append((a, h_sb))
            # MM2s for the oldest complete quad (its relus had 1-2 pairs
            # of MM1 streams to complete behind); sigmoid one cycle later
            if len(pend) >= 4:
                if sig_q is not None:
                    drain(sig_q)
                mm2_only(pend[:4])
                sig_q = pend[0][0] // 4
                pend = pend[4:]
            pend += new_pend
        while pend:
            if sig_q is not None:
                drain(sig_q)
            mm2_only(pend[:4])
            sig_q = pend[0][0] // 4
            pend = pend[4:]
        drain(sig_q)


def _install_ntff_hook():
    """Register the axon NTFF profile hook (normally provided by the agent
    image's antenv.axon_hooks). Needed only for trace=True runs."""
    import sys as _sys, types as _types, ctypes, contextlib

    if "antenv.axon_hooks" not in _sys.modules:
        mod = _types.ModuleType("antenv.axon_hooks")
        _h = [None]
        mod.set_axon_ntff_profile_hook = lambda h: _h.__setitem__(0, h)
        mod.get_axon_ntff_profile_hook = lambda: _h[0]
        _sys.modules["antenv.axon_hooks"] = mod
        try:
            import antenv
            antenv.axon_hooks = mod
        except ImportError:
            pass
    mod = _sys.modules["antenv.axon_hooks"]
    if mod.get_axon_ntff_profile_hook() is not None:
        return

    lib = ctypes.CDLL("/opt/axon/libaxon_pjrt.so")
    lib.axon_start_nrt_profile.argtypes = [
        ctypes.POINTER(ctypes.c_int64), ctypes.c_size_t]
    lib.axon_start_nrt_profile.restype = ctypes.c_int64
    lib.axon_stop_nrt_profile.argtypes = [ctypes.c_char_p]
    lib.axon_stop_nrt_profile.restype = ctypes.c_int64

    @contextlib.contextmanager
    def _hook(output_dir, device_ids):
        import jax
        jax.devices()
        if device_ids:
            ids = (ctypes.c_int64 * len(device_ids))(*device_ids)
            rc = lib.axon_start_nrt_profile(ids, len(device_ids))
        else:
            rc = lib.axon_start_nrt_profile(None, 0)
        if rc != 0:
            raise RuntimeError(f"axon_start_nrt_profile rc={rc}")
        try:
            yield
        finally:
            n = lib.axon_stop_nrt_profile(str(output_dir).encode())
            print(f"ntff profile: {n} file(s) -> {output_dir}")

    mod.set_axon_ntff_profile_hook(_hook)
    # artifact upload needs a bucket; stub it out for local profiling
    bass_utils.upload_artifacts = lambda tmpdir: f"local://{tmpdir}"


def kernel(x, W1, b1, W2, b2, trace=False):
    if "nc" not in _cached:
        _cached["nc"] = _build_nc()
    nc = _cached["nc"]
    if trace:
        try:
            _install_ntff_hook()
        except Exception as e:
            print("ntff hook install failed:", e)
            trace = False

    xt = np.ascontiguousarray(
        x.reshape(B, A * LAT).astype(ml_dtypes.bfloat16).T)     # [19968, 8192]
    w1h = np.ascontiguousarray(
        W1.reshape(NPAIR, 128, 128).transpose(1, 0, 2)).astype(
            ml_dtypes.bfloat16)                                  # [128,156,128]
    w2h = np.ascontiguousarray(
        W2.reshape(A, HID).T).astype(ml_dtypes.bfloat16)         # [128, 312]
    b1h = np.ascontiguousarray(b1.T).astype(np.float32)          # [128, 312]
    b2h = np.zeros((128, NQUAD), np.float32)
    b2h[::32, :] = b2.reshape(NQUAD, 4).T

    in_maps = []
    for c in range(NCORES):
        in_maps.append({
            "xt": np.ascontiguousarray(xt[:, c * BS:(c + 1) * BS]),
            "w1": w1h, "w2": w2h, "b1": b1h, "b2": b2h,
        })

    res = bass_utils.run_bass_kernel_spmd(
        nc, in_maps, core_ids=list(range(NCORES)), trace=trace)
    _cached["last_results"] = res

    out = np.empty((B, A), np.float32)
    for c in range(NCORES):
        out[c * BS:(c + 1) * BS, :] = res.results[c]["ot"].T
    return out


# revision 37
# speedup vs baseline: 1.0610x; 1.0610x over previous
"""Trainium2 Bass kernel for per-attribute MLP decoder (nn_AttrDecoder).

Computes, for each attribute a (A=312 independent blocks):
    h = relu(x[:, a*64:(a+1)*64] @ W1[a] + b1[a])      # [B, 128]
    o[:, a] = sigmoid(h @ W2[a] + b2[a])               # [B, 1]

Strategy:
  - Data-parallel over batch: B=8192 -> 1024 rows per core across 8 cores.
  - x is marshaled on the host to bf16 and transposed ([A*LAT, B]) so the
    contraction dim (LAT) lands on SBUF partitions as the tensor engine
    requires, and weights are pre-packed into PE-friendly layouts.
  - MM1: attributes in pairs; W1[2i] on PE rows 0-63, W1[2i+1] on rows
    64-127; two row-tiled K=64 matmuls run concurrently (tile_position
    (0,0)/(64,0)), N=512 batch columns, h^T accumulated in PSUM.
  - ReLU + b1 fused into the PSUM->SBUF copy (bf16 out), split across
    ScalarE (activation) and VectorE (tensor_scalar add+max) by throughput.
  - MM2: per attribute quad, four M=1 col-tiled matmuls (tile_position
    (0,32j), PSUM partitions {0,32,64,96} of a shared bank) so the four
    h-streams overlap on different PE column-groups.
  - Sigmoid + b2 on ScalarE per quad bank; strided-partition DMA stores
    rows 4q..4q+3 of the o^T output; host transposes during the gather.
"""

import numpy as np
import ml_dtypes

import concourse.bass as bass
import concourse.tile as tile
from concourse import mybir
from concourse import bass_utils

A = 312
LAT = 64
HID = 128
B = 8192
NCORES = 8
BS = B // NCORES          # 1024 batch rows per core
NPAIR = A // 2            # 156
NQUAD = A // 4            # 78
BT = 512                  # batch tile (one PSUM bank of fp32)
NBT = BS // BT            # 2

_cached = {}


def _legalize_waits(nc, max_waits=1):
    """Walrus in this toolchain encodes at most one sync-wait per instruction.
    Hoist extra waits onto standalone EventSemaphore instructions placed just
    before the owner on the same engine queue (queue order preserves the
    happens-before)."""
    nsplit = 0
    for bb in nc.m.functions[0].blocks:
        new_insts = []
        changed = False
        for inst in bb.instructions:
            si = getattr(inst, "sync_info", None)
            if si is not None and len(si.on_wait) > max_waits:
                waits = list(si.on_wait)
                for k, w in enumerate(waits[:-max_waits]):
                    es = mybir.InstEventSemaphore(name=f"{inst.name}-hw{k}")
                    es.engine = inst.engine
                    es.opcode = "EventSemaphore"
                    es.sync_info = mybir.SyncInfo(on_wait=[w], on_update=[])
                    new_insts.append(es)
                    nsplit += 1
                inst.sync_info = mybir.SyncInfo(
                    on_wait=waits[-max_waits:], on_update=list(si.on_update))
                changed = True
            new_insts.append(inst)
        if changed:
            bb.instructions = new_insts
    return nsplit


def _build_nc():
    nc = bass.Bass("TRN2", target_bir_lowering=False, debug=False,
                   num_devices=NCORES)
    xt = nc.dram_tensor("xt", [A * LAT, BS], mybir.dt.bfloat16,
                        kind="ExternalInput").ap()
    w1 = nc.dram_tensor("w1", [128, NPAIR, 128], mybir.dt.bfloat16,
                        kind="ExternalInput").ap()
    w2 = nc.dram_tensor("w2", [HID, A], mybir.dt.bfloat16,
                        kind="ExternalInput").ap()
    b1 = nc.dram_tensor("b1", [HID, A], mybir.dt.float32,
                        kind="ExternalInput").ap()
    b2 = nc.dram_tensor("b2", [128, NQUAD], mybir.dt.float32,
                        kind="ExternalInput").ap()
    ot = nc.dram_tensor("ot", [A, BS], mybir.dt.float32,
                        kind="ExternalOutput").ap()

    with tile.TileContext(nc, trace_sim=False) as tc:
        _body(tc, xt, w1, w2, b1, b2, ot)
    _legalize_waits(nc)
    return nc


def _body(tc, xt, w1, w2, b1, b2, ot):
    nc = tc.nc
    from contextlib import ExitStack
    with ExitStack() as ctx:
        singles = ctx.enter_context(tc.tile_pool(name="singles", bufs=1))
        xpool = ctx.enter_context(tc.tile_pool(name="x", bufs=6))
        hpool_a = ctx.enter_context(tc.tile_pool(name="ha", bufs=6))
        hpool_v = ctx.enter_context(tc.tile_pool(name="hv", bufs=6))
        opool = ctx.enter_context(tc.tile_pool(name="osb", bufs=3))
        hps = ctx.enter_context(
            tc.tile_pool(name="hps", bufs=3, space=bass.MemorySpace.PSUM))
        ops = ctx.enter_context(
            tc.tile_pool(name="ops", bufs=1, space=bass.MemorySpace.PSUM))

        # Resident weights/biases. w1 is 5MB: load in chunks so the first
        # pairs' matmuls don't stall on the whole transfer.
        w1_sb = singles.tile([128, NPAIR, 128], mybir.dt.bfloat16)
        W1CH = 12
        for c in range(0, NPAIR, W1CH):
            ce = min(c + W1CH, NPAIR)
            nc.scalar.dma_start(w1_sb[:, c:ce, :], w1[:, c:ce, :])
        w2_sb = singles.tile([HID, A], mybir.dt.bfloat16)
        nc.scalar.dma_start(w2_sb[:], w2[:])
        b1_sb = singles.tile([HID, A], mybir.dt.float32)
        nc.scalar.dma_start(b1_sb[:], b1[:])
        b2_sb = singles.tile([128, NQUAD], mybir.dt.float32)
        nc.scalar.dma_start(b2_sb[:], b2[:])

        o_ps = ops.tile([128, NBT, BT], mybir.dt.float32, name="o_bank")

        def mm2_only(quad):
            """Emit one quad's 8 MM2s, bt-outer so the 4 attrs' streams hit
            4 different PE column-groups back-to-back (concurrent)."""
            for bt in range(NBT):
                for a, h_sb in quad:
                    jj = a % 4
                    nc.tensor.matmul(
                        o_ps[32 * jj:32 * jj + 1, bt, :],
                        w2_sb[:, a:a + 1],
                        h_sb[:, bt, :],
                        start=True, stop=True,
                        tile_position=(0, 32 * jj),
                    )

        def drain(q):
            """Sigmoid + store for quad q, emitted a full pair after its
            MM2s so the sigmoid never blocks the ACT queue on them."""
            o_sb = opool.tile([128, NBT, BT], mybir.dt.float32, name="osb")
            nc.scalar.activation(
                out=o_sb[:], in_=o_ps[:],
                func=mybir.ActivationFunctionType.Sigmoid,
                bias=b2_sb[:, q:q + 1], scale=1.0)
            nc.sync.dma_start(
                out=ot[4 * q:4 * q + 4, :].rearrange(
                    "p (n b) -> p n b", n=NBT),
                in_=o_sb[::32, :, :])

        pend = []           # (a, h_sb) entries not yet MM2'd
        sig_q = None        # quad whose sigmoid/store is deferred
        for p in range(NPAIR):
            x_tile = xpool.tile([128, BS], mybir.dt.bfloat16)
            nc.sync.dma_start(out=x_tile[:],
                              in_=xt[p * 128:(p + 1) * 128, :])
            h_pss = [hps.tile([128, NBT, BT], mybir.dt.float32, name="hps"),
                     hps.tile([128, NBT, BT], mybir.dt.float32, name="hps")]
            # MM1s: interleave the two attrs (disjoint PE row groups) so
            # their streams overlap; bt-halves of one attr are sequential.
            for bt in range(NBT):
                for j in range(2):
                    nc.tensor.matmul(
                        h_pss[j][:, bt, :],
                        w1_sb[j * 64:(j + 1) * 64, p, :],
                        x_tile[j * 64:(j + 1) * 64, bass.ds(bt * BT, BT)],
                        start=True, stop=True,
                        tile_position=(j * 64, 0),
                    )
            # relu: one FD=1024 op per attr, alternating engines
            new_pend = []
            for j in range(2):
                a = 2 * p + j
                use_act = (a % 2 == 0) and (a % 12 != 0)
                hp = hpool_a if use_act else hpool_v
                h_sb = hp.tile([HID, NBT, BT], mybir.dt.bfloat16,
                               name="hsb")
                if use_act:
                    nc.scalar.activation(
                        out=h_sb[:], in_=h_pss[j][:],
                        func=mybir.ActivationFunctionType.Relu,
                        bias=b1_sb[:, a:a + 1], scale=1.0)
                else:
                    nc.vector.tensor_scalar(
                        out=h_sb[:], in0=h_pss[j][:],
                        scalar1=b1_sb[:, a:a + 1], scalar2=0.0,
                        op0=mybir.AluOpType.add,
                        op1=mybir.AluOpType.max)
                new_pend.append((a, h_sb))
            # MM2s for the oldest complete quad (its relus had 1-2 pairs
            # of MM1 streams to complete behind); sigmoid one cycle later
            if len(pend) >= 4:
                if sig_q is not None:
                    drain(sig_q)
                mm2_only(pend[:4])
                sig_q = pend[0][0] // 4
                pend = pend[4:]
            pend += new_pend
        while pend:
            if sig_q is not None:
                drain(sig_q)
            mm2_only(pend[:4])
            sig_q = pend[0][0] // 4
            pend = pend[4:]
        drain(sig_q)


def _install_ntff_hook():
    """Register the axon NTFF profile hook (normally provided by the agent
    image's antenv.axon_hooks). Needed only for trace=True runs."""
    import sys as _sys, types as _types, ctypes, contextlib

    if "antenv.axon_hooks" not in _sys.modules:
        mod = _types.ModuleType("antenv.axon_hooks")
        _h = [None]
        mod.set_axon_ntff_profile_hook = lambda h: _h.__setitem__(0, h)
        mod.get_axon_ntff_profile_hook = lambda: _h[0]
        _sys.modules["antenv.axon_hooks"] = mod
        try:
            import antenv
            antenv.axon_hooks = mod
        except ImportError:
            pass
    mod = _sys.modules["antenv.axon_hooks"]
    if mod.get_axon_ntff_profile_hook() is not None:
        return

    lib = ctypes.CDLL("/opt/axon/libaxon_pjrt.so")
    lib.axon_start_nrt_profile.argtypes = [
        ctypes.POINTER(ctypes.c_int64), ctypes.c_size_t]
    lib.axon_start_nrt_profile.restype = ctypes.c_int64
    lib.axon_stop_nrt_profile.argtypes = [ctypes.c_char_p]
    lib.axon_stop_nrt_profile.restype = ctypes.c_int64

    @contextlib.contextmanager
    def _hook(output_dir, device_ids):
        import jax
        jax.devices()
        if device_ids:
            ids = (ctypes.c_int64 * len(device_ids))(*device_ids)
            rc = lib.axon_start_nrt_profile(ids, len(device_ids))
        else:
            rc = lib.axon_start_nrt_profile(None, 0)
        if rc != 0:
            raise RuntimeError(f"axon_start_nrt_profile rc={rc}")
        try:
            yield
        finally:
            n = lib.axon_stop_nrt_profile(str(output_dir).encode())
            print(f"ntff profile: {n} file(s) -> {output_dir}")

    mod.set_axon_ntff_profile_hook(_hook)
    # artifact upload needs a bucket; stub it out for local profiling
    bass_utils.upload_artifacts = lambda tmpdir: f"local://{tmpdir}"


def kernel(x, W1, b1, W2, b2, trace=False):
    if "nc" not in _cached:
        _cached["nc"] = _build_nc()
    nc = _cached["nc"]
    if trace:
        try:
            _install_ntff_hook()
        except Exception as e:
            print("ntff hook install failed:", e)
            trace = False

    xt = np.ascontiguousarray(
        x.reshape(B, A * LAT).astype(ml_dtypes.bfloat16).T)     # [19968, 8192]
    w1h = np.ascontiguousarray(
        W1.reshape(NPAIR, 128, 128).transpose(1, 0, 2)).astype(
            ml_dtypes.bfloat16)                                  # [128,156,128]
    w2h = np.ascontiguousarray(
        W2.reshape(A, HID).T).astype(ml_dtypes.bfloat16)         # [128, 312]
    b1h = np.ascontiguousarray(b1.T).astype(np.float32)          # [128, 312]
    b2h = np.zeros((128, NQUAD), np.float32)
    b2h[::32, :] = b2.reshape(NQUAD, 4).T

    in_maps = []
    for c in range(NCORES):
        in_maps.append({
            "xt": np.ascontiguousarray(xt[:, c * BS:(c + 1) * BS]),
            "w1": w1h, "w2": w2h, "b1": b1h, "b2": b2h,
        })

    res = bass_utils.run_bass_kernel_spmd(
        nc, in_maps, core_ids=list(range(NCORES)), trace=trace)
    _cached["last_results"] = res

    out = np.empty((B, A), np.float32)
    for c in range(NCORES):
        out[c * BS:(c + 1) * BS, :] = res.results[c]["ot"].T
    return out


# revision 38
# speedup vs baseline: 1.0784x; 1.0164x over previous
"""Trainium2 Bass kernel for per-attribute MLP decoder (nn_AttrDecoder).

Computes, for each attribute a (A=312 independent blocks):
    h = relu(x[:, a*64:(a+1)*64] @ W1[a] + b1[a])      # [B, 128]
    o[:, a] = sigmoid(h @ W2[a] + b2[a])               # [B, 1]

Strategy:
  - Data-parallel over batch: B=8192 -> 1024 rows per core across 8 cores.
  - x is marshaled on the host to bf16 and transposed ([A*LAT, B]) so the
    contraction dim (LAT) lands on SBUF partitions as the tensor engine
    requires, and weights are pre-packed into PE-friendly layouts.
  - MM1: attributes in pairs; W1[2i] on PE rows 0-63, W1[2i+1] on rows
    64-127; two row-tiled K=64 matmuls run concurrently (tile_position
    (0,0)/(64,0)), N=512 batch columns, h^T accumulated in PSUM.
  - ReLU + b1 fused into the PSUM->SBUF copy (bf16 out), split across
    ScalarE (activation) and VectorE (tensor_scalar add+max) by throughput.
  - MM2: per attribute quad, four M=1 col-tiled matmuls (tile_position
    (0,32j), PSUM partitions {0,32,64,96} of a shared bank) so the four
    h-streams overlap on different PE column-groups.
  - Sigmoid + b2 on ScalarE per quad bank; strided-partition DMA stores
    rows 4q..4q+3 of the o^T output; host transposes during the gather.
"""

import numpy as np
import ml_dtypes

import concourse.bass as bass
import concourse.tile as tile
from concourse import mybir
from concourse import bass_utils

A = 312
LAT = 64
HID = 128
B = 8192
NCORES = 8
BS = B // NCORES          # 1024 batch rows per core
NPAIR = A // 2            # 156
NQUAD = A // 4            # 78
BT = 512                  # batch tile (one PSUM bank of fp32)
NBT = BS // BT            # 2

_cached = {}


def _legalize_waits(nc, max_waits=1):
    """Walrus in this toolchain encodes at most one sync-wait per instruction.
    Hoist extra waits onto standalone EventSemaphore instructions placed just
    before the owner on the same engine queue (queue order preserves the
    happens-before)."""
    nsplit = 0
    for bb in nc.m.functions[0].blocks:
        new_insts = []
        changed = False
        for inst in bb.instructions:
            si = getattr(inst, "sync_info", None)
            if si is not None and len(si.on_wait) > max_waits:
                waits = list(si.on_wait)
                for k, w in enumerate(waits[:-max_waits]):
                    es = mybir.InstEventSemaphore(name=f"{inst.name}-hw{k}")
                    es.engine = inst.engine
                    es.opcode = "EventSemaphore"
                    es.sync_info = mybir.SyncInfo(on_wait=[w], on_update=[])
                    new_insts.append(es)
                    nsplit += 1
                inst.sync_info = mybir.SyncInfo(
                    on_wait=waits[-max_waits:], on_update=list(si.on_update))
                changed = True
            new_insts.append(inst)
        if changed:
            bb.instructions = new_insts
    return nsplit


def _build_nc():
    nc = bass.Bass("TRN2", target_bir_lowering=False, debug=False,
                   num_devices=NCORES)
    xt = nc.dram_tensor("xt", [A * LAT, BS], mybir.dt.bfloat16,
                        kind="ExternalInput").ap()
    w1 = nc.dram_tensor("w1", [128, NPAIR, 128], mybir.dt.bfloat16,
                        kind="ExternalInput").ap()
    w2 = nc.dram_tensor("w2", [HID, A], mybir.dt.bfloat16,
                        kind="ExternalInput").ap()
    b1 = nc.dram_tensor("b1", [HID, A], mybir.dt.float32,
                        kind="ExternalInput").ap()
    b2 = nc.dram_tensor("b2", [128, NQUAD], mybir.dt.float32,
                        kind="ExternalInput").ap()
    ot = nc.dram_tensor("ot", [A, BS], mybir.dt.float32,
                        kind="ExternalOutput").ap()

    with tile.TileContext(nc, trace_sim=False) as tc:
        _body(tc, xt, w1, w2, b1, b2, ot)
    _legalize_waits(nc)
    return nc


def _body(tc, xt, w1, w2, b1, b2, ot):
    nc = tc.nc
    from contextlib import ExitStack
    with ExitStack() as ctx:
        singles = ctx.enter_context(tc.tile_pool(name="singles", bufs=1))
        xpool = ctx.enter_context(tc.tile_pool(name="x", bufs=6))
        hpool_a = ctx.enter_context(tc.tile_pool(name="ha", bufs=8))
        hpool_v = ctx.enter_context(tc.tile_pool(name="hv", bufs=8))
        opool = ctx.enter_context(tc.tile_pool(name="osb", bufs=4))
        hps = ctx.enter_context(
            tc.tile_pool(name="hps", bufs=3, space=bass.MemorySpace.PSUM))
        ops = ctx.enter_context(
            tc.tile_pool(name="ops", bufs=1, space=bass.MemorySpace.PSUM))

        # Resident weights/biases. Small tensors (biases, W2) first: the
        # first relu needs b1 immediately. w1 is 5MB: chunked so the first
        # pairs' matmuls only wait for their own slice.
        b1_sb = singles.tile([HID, A], mybir.dt.float32)
        nc.scalar.dma_start(b1_sb[:], b1[:])
        w2_sb = singles.tile([HID, A], mybir.dt.bfloat16)
        nc.scalar.dma_start(w2_sb[:], w2[:])
        b2_sb = singles.tile([128, NQUAD], mybir.dt.float32)
        nc.scalar.dma_start(b2_sb[:], b2[:])
        w1_sb = singles.tile([128, NPAIR, 128], mybir.dt.bfloat16)
        W1CH = 12
        for c in range(0, NPAIR, W1CH):
            ce = min(c + W1CH, NPAIR)
            nc.scalar.dma_start(w1_sb[:, c:ce, :], w1[:, c:ce, :])

        o_ps = ops.tile([128, NBT, BT], mybir.dt.float32, name="o_bank")

        def mm2_only(quad):
            """Emit one quad's 8 MM2s, bt-outer so the 4 attrs' streams hit
            4 different PE column-groups back-to-back (concurrent)."""
            for bt in range(NBT):
                for a, h_sb in quad:
                    jj = a % 4
                    nc.tensor.matmul(
                        o_ps[32 * jj:32 * jj + 1, bt, :],
                        w2_sb[:, a:a + 1],
                        h_sb[:, bt, :],
                        start=True, stop=True,
                        tile_position=(0, 32 * jj),
                    )

        def drain(q):
            """Sigmoid + store for quad q, emitted a full pair after its
            MM2s so the sigmoid never blocks the ACT queue on them."""
            o_sb = opool.tile([128, NBT, BT], mybir.dt.float32, name="osb")
            nc.scalar.activation(
                out=o_sb[:], in_=o_ps[:],
                func=mybir.ActivationFunctionType.Sigmoid,
                bias=b2_sb[:, q:q + 1], scale=1.0)
            nc.sync.dma_start(
                out=ot[4 * q:4 * q + 4, :].rearrange(
                    "p (n b) -> p n b", n=NBT),
                in_=o_sb[::32, :, :])

        pend = []           # (a, h_sb) entries not yet MM2'd
        sig_q = None        # quad whose sigmoid/store is deferred
        for p in range(NPAIR):
            x_tile = xpool.tile([128, BS], mybir.dt.bfloat16)
            nc.sync.dma_start(out=x_tile[:],
                              in_=xt[p * 128:(p + 1) * 128, :])
            h_pss = [hps.tile([128, NBT, BT], mybir.dt.float32, name="hps"),
                     hps.tile([128, NBT, BT], mybir.dt.float32, name="hps")]
            # MM1s: interleave the two attrs (disjoint PE row groups) so
            # their streams overlap; bt-halves of one attr are sequential.
            for bt in range(NBT):
                for j in range(2):
                    nc.tensor.matmul(
                        h_pss[j][:, bt, :],
                        w1_sb[j * 64:(j + 1) * 64, p, :],
                        x_tile[j * 64:(j + 1) * 64, bass.ds(bt * BT, BT)],
                        start=True, stop=True,
                        tile_position=(j * 64, 0),
                    )
            # relu: one FD=1024 op per attr, alternating engines
            new_pend = []
            for j in range(2):
                a = 2 * p + j
                use_act = (a % 2 == 0) and (a % 12 != 0)
                hp = hpool_a if use_act else hpool_v
                h_sb = hp.tile([HID, NBT, BT], mybir.dt.bfloat16,
                               name="hsb")
                if use_act:
                    nc.scalar.activation(
                        out=h_sb[:], in_=h_pss[j][:],
                        func=mybir.ActivationFunctionType.Relu,
                        bias=b1_sb[:, a:a + 1], scale=1.0)
                else:
                    nc.vector.tensor_scalar(
                        out=h_sb[:], in0=h_pss[j][:],
                        scalar1=b1_sb[:, a:a + 1], scalar2=0.0,
                        op0=mybir.AluOpType.add,
                        op1=mybir.AluOpType.max)
                new_pend.append((a, h_sb))
            # MM2s for the oldest complete quad (its relus had 1-2 pairs
            # of MM1 streams to complete behind); sigmoid one cycle later
            if len(pend) >= 4:
                if sig_q is not None:
                    drain(sig_q)
                mm2_only(pend[:4])
                sig_q = pend[0][0] // 4
                pend = pend[4:]
            pend += new_pend
        while pend:
            if sig_q is not None:
                drain(sig_q)
            mm2_only(pend[:4])
            sig_q = pend[0][0] // 4
            pend = pend[4:]
        drain(sig_q)


def _install_ntff_hook():
    """Register the axon NTFF profile hook (normally provided by the agent
    image's antenv.axon_hooks). Needed only for trace=True runs."""
    import sys as _sys, types as _types, ctypes, contextlib

    if "antenv.axon_hooks" not in _sys.modules:
        mod = _types.ModuleType("antenv.axon_hooks")
        _h = [None]
        mod.set_axon_ntff_profile_hook = lambda h: _h.__setitem__(0, h)
        mod.get_axon_ntff_profile_hook = lambda: _h[0]
        _sys.modules["antenv.axon_hooks"] = mod
        try:
            import antenv
            antenv.axon_hooks = mod
        except ImportError:
            pass
    mod = _sys.modules["antenv.axon_hooks"]
    if mod.get_axon_ntff_profile_hook() is not None:
        return

    lib = ctypes.CDLL("/opt/axon/libaxon_pjrt.so")
    lib.axon_start_nrt_profile.argtypes = [
        ctypes.POINTER(ctypes.c_int64), ctypes.c_size_t]
    lib.axon_start_nrt_profile.restype = ctypes.c_int64
    lib.axon_stop_nrt_profile.argtypes = [ctypes.c_char_p]
    lib.axon_stop_nrt_profile.restype = ctypes.c_int64

    @contextlib.contextmanager
    def _hook(output_dir, device_ids):
        import jax
        jax.devices()
        if device_ids:
            ids = (ctypes.c_int64 * len(device_ids))(*device_ids)
            rc = lib.axon_start_nrt_profile(ids, len(device_ids))
        else:
            rc = lib.axon_start_nrt_profile(None, 0)
        if rc != 0:
            raise RuntimeError(f"axon_start_nrt_profile rc={rc}")
        try:
            yield
        finally:
            n = lib.axon_stop_nrt_profile(str(output_dir).encode())
            print(f"ntff profile: {n} file(s) -> {output_dir}")

    mod.set_axon_ntff_profile_hook(_hook)
    # artifact upload needs a bucket; stub it out for local profiling
    bass_utils.upload_artifacts = lambda tmpdir: f"local://{tmpdir}"


def kernel(x, W1, b1, W2, b2, trace=False):
    if "nc" not in _cached:
        _cached["nc"] = _build_nc()
    nc = _cached["nc"]
    if trace:
        try:
            _install_ntff_hook()
        except Exception as e:
            print("ntff hook install failed:", e)
            trace = False

    xt = np.ascontiguousarray(
        x.reshape(B, A * LAT).astype(ml_dtypes.bfloat16).T)     # [19968, 8192]
    w1h = np.ascontiguousarray(
        W1.reshape(NPAIR, 128, 128).transpose(1, 0, 2)).astype(
            ml_dtypes.bfloat16)                                  # [128,156,128]
    w2h = np.ascontiguousarray(
        W2.reshape(A, HID).T).astype(ml_dtypes.bfloat16)         # [128, 312]
    b1h = np.ascontiguousarray(b1.T).astype(np.float32)          # [128, 312]
    b2h = np.zeros((128, NQUAD), np.float32)
    b2h[::32, :] = b2.reshape(NQUAD, 4).T

    in_maps = []
    for c in range(NCORES):
        in_maps.append({
            "xt": np.ascontiguousarray(xt[:, c * BS:(c + 1) * BS]),
            "w1": w1h, "w2": w2h, "b1": b1h, "b2": b2h,
        })

    res = bass_utils.run_bass_kernel_spmd(
        nc, in_maps, core_ids=list(range(NCORES)), trace=trace)
    _cached["last_results"] = res

    out = np.empty((B, A), np.float32)
    for c in range(NCORES):
        out[c * BS:(c + 1) * BS, :] = res.results[c]["ot"].T
    return out
